# revision 11
# baseline (speedup 1.0000x reference)
"""Trainium2 Bass kernel for a dense transformer block (B=4,T=2048,C=1024,H=16,F=4096).

Sharding (8 cores, no collectives): core 2s+p owns sequence s (p=parity).
Each core handles two 512-token q-chunks; the host PERMUTES the sequence
per-core so the q-chunks always sit at permuted positions [0:512] and
[1536:2048] (parity 0: chunks (0,3) with identity perm; parity 1: chunks
(1,2) with perm [1,0,3,2]).  That makes every operand offset uniform across
cores (single SPMD program); causality lives entirely in per-core mask data.

Attention computes scores pre-transposed (S^T[k,q] = K Q^T) with the two
heads of a 128-partition pair issued as back-to-back row-tiled matmuls
(tile_position (0,0)/(64,0)) so they run concurrently in the PE array.
exp is unnormalized; the denominator comes from a ones-column appended to V
and is partition-broadcast via a tiny fp32 matmul (no DRAM roundtrip).
Causal masks are multiplied in on the GPSIMD engine.  Dummy "tickle"
matmuls tied to exp outputs keep the PE HAM clock warm through the
scalar-bound attention phase.
"""
import sys, types
import numpy as np
import ml_dtypes

# --- make the NTFF profile hook importable (missing module in this image) ---
def _install_hooks():
    try:
        import antenv
        if "antenv.axon_hooks" not in sys.modules:
            m = types.ModuleType("antenv.axon_hooks")
            m._hook = None
            m.set_axon_ntff_profile_hook = lambda h: setattr(m, "_hook", h)
            m.get_axon_ntff_profile_hook = lambda: m._hook
            sys.modules["antenv.axon_hooks"] = m
            antenv.axon_hooks = m
    except Exception:
        pass
_install_hooks()

import concourse.bass as bass
import concourse.tile as tile
from concourse import mybir, bacc
from concourse.bass_utils import run_bass_kernel_spmd

BF16 = mybir.dt.bfloat16
F32 = mybir.dt.float32
bfloat16 = ml_dtypes.bfloat16

T, C, H, D, F = 2048, 1024, 16, 64, 4096
CH = 512            # q-chunk width
P = 128
NT = T // P         # 16 token tiles
NC8 = C // P        # 8 contraction tiles
NKT = (8, 16)       # kt-tile slots per (ci=0, ci=1)
QOFF = (0, 3 * CH)  # permuted-seq offsets of the two q-chunks

_cache = {}


def build_nc(add_bfc1, add_bfc2):
    nc = bacc.Bacc()
    x_seq = nc.declare_dram_parameter("x_seq", [T, C], F32, isOutput=False)
    w_attn = nc.declare_dram_parameter("w_attn", [C, 3 * C], BF16, isOutput=False)
    w_proj = nc.declare_dram_parameter("w_proj", [C, C], BF16, isOutput=False)
    w_fc1 = nc.declare_dram_parameter("w_fc1", [C, F], BF16, isOutput=False)
    w_fc2 = nc.declare_dram_parameter("w_fc2", [F, C], BF16, isOutput=False)
    bfc1 = nc.declare_dram_parameter("bfc1_col", [P, F // P], F32, isOutput=False)
    bfc2 = nc.declare_dram_parameter("bfc2_rep", [P, C], F32, isOutput=False)
    ident = nc.declare_dram_parameter("ident", [P, P], BF16, isOutput=False)
    maskA = nc.declare_dram_parameter("maskA", [P, 8, CH], BF16, isOutput=False)
    maskB = nc.declare_dram_parameter("maskB", [P, 8, CH], BF16, isOutput=False)
    out = nc.declare_dram_parameter("out", [2 * CH, C], F32, isOutput=True)

    wAr = w_attn.rearrange("(co p) f -> p co f", p=P)
    wPr = w_proj.rearrange("(co p) f -> p co f", p=P)
    w1r = w_fc1.rearrange("(co p) f -> p co f", p=P)
    w2r = w_fc2.rearrange("(fo p) c -> p fo c", p=P)

    ec = [0]  # evict-engine round robin

    with tile.TileContext(nc, pool_alloc_mode="queue") as tc:
        with tc.tile_pool(name="consts", bufs=1) as consts:
            id_sb = consts.tile([P, P], BF16)
            nc.sync.dma_start(out=id_sb[:], in_=ident[:])
            eps_sb = consts.tile([P, 1], F32)
            nc.vector.memset(eps_sb[:], 1e-5)
            ones_sb = consts.tile([P, D], F32)
            nc.vector.memset(ones_sb[:], 1.0)
            bfc1_sb = bfc2_sb = None
            if add_bfc1:
                bfc1_sb = consts.tile([P, F // P], F32)
                nc.sync.dma_start(out=bfc1_sb[:], in_=bfc1[:])
            if add_bfc2:
                bfc2_sb = consts.tile([P, C], F32)
                nc.sync.dma_start(out=bfc2_sb[:], in_=bfc2[:])

            def copy_evict(dst, src):
                if ec[0] % 3 == 0:
                    nc.vector.tensor_copy(out=dst, in_=src)
                else:
                    nc.scalar.copy(out=dst, in_=src)
                ec[0] += 1

            def ln_common(src_ap, pool, tagp):
                st = pool.tile([P, 2, 6], F32, tag=tagp + "st")
                xr = src_ap.rearrange("p (s d) -> p s d", s=2)
                for s in range(2):
                    nc.vector.bn_stats(out=st[:, s, :], in_=xr[:, s, :])
                mv = pool.tile([P, 2], F32, tag=tagp + "mv")
                nc.vector.bn_aggr(out=mv[:], in_=st[:])
                rstd = pool.tile([P, 1], F32, tag=tagp + "rs")
                nc.scalar.activation(out=rstd[:], in_=mv[:, 1:2],
                                     func=mybir.ActivationFunctionType.Sqrt,
                                     bias=eps_sb[:], scale=1.0)
                nc.vector.reciprocal(out=rstd[:], in_=rstd[:])
                h_t = pool.tile([P, C], BF16, tag=tagp + "ht")
                nc.vector.tensor_scalar(
                    out=h_t[:], in0=src_ap, scalar1=mv[:, 0:1],
                    scalar2=rstd[:], op0=mybir.AluOpType.subtract,
                    op1=mybir.AluOpType.mult)
                return h_t

            _cm_h2T = tc.tile_pool(name="p_h2T", bufs=1)
            p_h2T = _cm_h2T.__enter__()
            h2T_sb = p_h2T.tile([P, NC8, 2 * CH], BF16)
            _cm_dram = tc.tile_pool(name="p_dram", bufs=1, space="DRAM")
            p_dram = _cm_dram.__enter__()
            xo_dram = p_dram.tile([2 * CH, C], BF16)
            _cm_yT = tc.tile_pool(name="p_yT", bufs=1)
            p_yT = _cm_yT.__enter__()
            yT_sb = p_yT.tile([P, NC8, 2 * CH], BF16)
            _cm_kqv = tc.tile_pool(name="p_kqv", bufs=1)
            p_kqv = _cm_kqv.__enter__()
            if True:
                kT_sb = p_kqv.tile([P, NC8, T], BF16)       # [d(2 heads)][hp][t]
                qT_sb = p_kqv.tile([P, NC8, 2 * CH], BF16)
                v_sb = p_kqv.tile([P, NT, H, D + 1], BF16)  # ones col at [..,64]
                nc.vector.memset(v_sb[:, :, :, D:D + 1], 1.0)

                # ================= P1: ln1 + transpose + QKV =================
                with tc.tile_pool(name="p_wA", bufs=1) as p_wA, \
                     tc.tile_pool(name="p_hT", bufs=2) as p_hT, \
                     tc.tile_pool(name="p_ln", bufs=2) as p_ln, \
                     tc.tile_pool(name="ps_tr", bufs=2, space="PSUM") as ps_tr, \
                     tc.tile_pool(name="ps_mm", bufs=5, space="PSUM") as ps_mm:
                    wA_sb = p_wA.tile([P, NC8, 3 * C], BF16)
                    # K cols first (first use), then V, then Q; split so x-tile
                    # DMAs aren't stuck behind one 6MB transfer
                    for c0 in (C, 3 * C // 2, 2 * C, 5 * C // 2, 0, C // 2):
                        nc.gpsimd.dma_start(out=wA_sb[:, :, c0:c0 + C // 2],
                                            in_=wAr[:, :, c0:c0 + C // 2])
                    # hT is a rolling per-chunk transposed-activation buffer;
                    # the two q-chunks are exactly perm chunks 0 and 3.
                    hTcs = {}

                    def get_hTc(tcx):
                        if tcx not in hTcs:
                            hTcs[tcx] = p_hT.tile([P, NC8, CH], BF16, tag="hTc",
                                                  name=f"hTc{tcx}")
                        return hTcs[tcx]

                    def ln_unit(tt):
                        hTc = get_hTc(tt // 4)
                        t4 = tt % 4
                        x_t = p_ln.tile([P, C], F32, tag="xt")
                        nc.sync.dma_start(out=x_t[:], in_=x_seq[tt * P:(tt + 1) * P, :])
                        h1_t = ln_common(x_t[:], p_ln, "l1")
                        for co in range(NC8):
                            tp = ps_tr.tile([P, P], BF16)
                            nc.tensor.transpose(tp[:], h1_t[:, co * P:(co + 1) * P], id_sb[:])
                            copy_evict(hTc[:, co, t4 * P:(t4 + 1) * P], tp[:])

                    def k_unit(tcx, hp):
                        hTc = hTcs[tcx]
                        pm = ps_mm.tile([P, CH], F32)
                        for ct in range(NC8):
                            nc.tensor.matmul(
                                pm[:], wA_sb[:, ct, C + hp * P:C + (hp + 1) * P],
                                hTc[:, ct, :],
                                start=(ct == 0), stop=(ct == NC8 - 1))
                        copy_evict(kT_sb[:, hp, tcx * CH:(tcx + 1) * CH], pm[:])

                    def v_unit(tcx, tt4, fh):
                        hTc = hTcs[tcx]
                        tt = tcx * 4 + tt4
                        pm = ps_mm.tile([P, CH], F32)
                        for ct in range(NC8):
                            nc.tensor.matmul(
                                pm[:], hTc[:, ct, tt4 * P:(tt4 + 1) * P],
                                wA_sb[:, ct, 2 * C + fh * CH:2 * C + (fh + 1) * CH],
                                start=(ct == 0), stop=(ct == NC8 - 1))
                        nc.vector.tensor_copy(
                            out=v_sb[:, tt, fh * 8:(fh + 1) * 8, 0:D],
                            in_=pm.rearrange("p (h d) -> p h d", d=D))

                    def q_unit(ci, hp):
                        hTc = hTcs[0 if ci == 0 else 3]
                        pm = ps_mm.tile([P, CH], F32)
                        for ct in range(NC8):
                            nc.tensor.matmul(
                                pm[:], wA_sb[:, ct, hp * P:(hp + 1) * P],
                                hTc[:, ct, :],
                                start=(ct == 0), stop=(ct == NC8 - 1))
                        copy_evict(qT_sb[:, hp, ci * CH:(ci + 1) * CH], pm[:])

                    for tt in range(4):
                        ln_unit(tt)
                    for tcx in range(4):
                        units = [lambda hp=hp: k_unit(tcx, hp) for hp in range(NC8)]
                        units += [lambda tt4=tt4, fh=fh: v_unit(tcx, tt4, fh)
                                  for tt4 in range(4) for fh in range(2)]
                        if tcx == 0:
                            units += [lambda hp=hp: q_unit(0, hp) for hp in range(NC8)]
                        nxt = list(range(4 * (tcx + 1), min(4 * (tcx + 2), NT)))
                        for i, u in enumerate(units):
                            u()
                            if i % 5 == 4 and nxt:
                                ln_unit(nxt.pop(0))
                        for tt in nxt:
                            ln_unit(tt)
                    for hp in range(NC8):
                        q_unit(1, hp)

                # ================= P2: attention =================
                with tc.tile_pool(name="p_mA", bufs=1) as p_mA, \
                     tc.tile_pool(name="p_mB", bufs=1) as p_mB, \
                     tc.tile_pool(name="p_pt", bufs=2) as p_pt, \
                     tc.tile_pool(name="p_aw", bufs=2) as p_aw, \
                     tc.tile_pool(name="ps_sc", bufs=2, space="PSUM") as ps_sc, \
                     tc.tile_pool(name="ps_av", bufs=2, space="PSUM") as ps_av, \
                     tc.tile_pool(name="ps_bc", bufs=1, space="PSUM") as ps_bc, \
                     tc.tile_pool(name="ps_tk", bufs=1, space="PSUM") as ps_tk:
                    mA_sb = p_mA.tile([P, 8, CH], BF16)
                    nc.sync.dma_start(out=mA_sb[:], in_=maskA[:])
                    mB_sb = p_mB.tile([P, 8, CH], BF16)
                    nc.sync.dma_start(out=mB_sb[:], in_=maskB[:])
                    tick = ps_tk.tile([1, D], F32)

                    pairs = [(ci, hp) for ci in range(2) for hp in range(NC8)]
                    state = {}

                    def tickle(prev_pt, g):
                        # tiny matmul reading a just-exp'd pt slice: keeps HAM warm
                        nc.tensor.matmul(tick[:], id_sb[:, 0:1],
                                         prev_pt[:, g % 8, 0, 0:D],
                                         start=True, stop=True)

                    def sxm_unit(pi):
                        ci, hp = pairs[pi]
                        n_kt = NKT[ci]
                        pt = p_pt.tile([P, NKT[1], 2, CH], BF16, tag="pt")
                        prev = state.get(pi - 1)
                        for g in range(n_kt):
                            sp = ps_sc.tile([P, 2, CH], F32)
                            for h2 in range(2):
                                nc.tensor.matmul(
                                    sp[:, h2, :],
                                    kT_sb[h2 * D:(h2 + 1) * D, hp, g * P:(g + 1) * P],
                                    qT_sb[h2 * D:(h2 + 1) * D, hp, ci * CH:(ci + 1) * CH],
                                    start=True, stop=True,
                                    tile_position=(h2 * D, 0))
                            if prev is not None and g % 2 == 0:
                                tickle(prev, g)
                            nc.scalar.activation(
                                pt[:, g, :, :], sp[:],
                                mybir.ActivationFunctionType.Exp, scale=0.125)
                        for h2 in range(2):
                            if ci == 0:
                                nc.vector.tensor_mul(out=pt[:, 0:8, h2, :],
                                                     in0=pt[:, 0:8, h2, :], in1=mA_sb[:])
                            else:
                                nc.vector.tensor_mul(out=pt[:, 8:16, h2, :],
                                                     in0=pt[:, 8:16, h2, :], in1=mB_sb[:])
                        state[pi] = pt

                    def av_unit(pi, h2):
                        ci, hp = pairs[pi]
                        n_kt = NKT[ci]
                        pt = state[pi]
                        ap = ps_av.tile([D + 1, CH], F32)
                        for kt in range(n_kt):
                            nc.tensor.matmul(
                                ap[:], v_sb[:, kt, 2 * hp + h2, :], pt[:, kt, h2, :],
                                start=(kt == 0), stop=(kt == n_kt - 1))
                        dnS = p_aw.tile([D + 1, CH], F32, tag="dn")
                        nc.vector.tensor_copy(out=dnS[D:D + 1, :], in_=ap[D:D + 1, :])
                        bc = ps_bc.tile([D, CH], F32)
                        nc.tensor.matmul(bc[:], ones_sb[D:D + 1, 0:D], dnS[D:D + 1, :],
                                         start=True, stop=True)
                        rc = p_aw.tile([D, CH], F32, tag="rc")
                        nc.vector.reciprocal_approx_fast(out=rc[:], in_=bc[:])
                        if h2 == 0:
                            nc.vector.tensor_mul(
                                out=yT_sb[0:D, hp, ci * CH:(ci + 1) * CH],
                                in0=ap[0:D, :], in1=rc[:])
                        else:
                            yt = p_aw.tile([D, CH], BF16, tag="ytmp")
                            nc.vector.tensor_mul(out=yt[:], in0=ap[0:D, :], in1=rc[:])
                            nc.sync.dma_start(
                                out=yT_sb[D:P, hp, ci * CH:(ci + 1) * CH], in_=yt[:])

                    # software pipeline: scores/exp/mask leads AV by one pair
                    for pi in range(len(pairs) + 1):
                        if pi < len(pairs):
                            sxm_unit(pi)
                        if pi >= 1:
                            av_unit(pi - 1, 0)
                            av_unit(pi - 1, 1)
                            state.pop(pi - 2, None)

                # ============ proj + residual + ln2 ============
                _cm_kqv.__exit__(None, None, None)
                with tc.tile_pool(name="p_wp", bufs=1) as p_wp, \
                     tc.tile_pool(name="p_pw", bufs=2) as p_pw, \
                     tc.tile_pool(name="ps_pr", bufs=3, space="PSUM") as ps_pr, \
                     tc.tile_pool(name="ps_t2", bufs=2, space="PSUM") as ps_t2:
                    wp_sb = p_wp.tile([P, NC8, C], BF16)
                    nc.sync.dma_start(out=wp_sb[:], in_=wPr[:])

                    def proj_unit(qt):
                        x_t = p_pw.tile([P, C], F32, tag="xq")
                        roff = QOFF[qt // 4] + (qt % 4) * P
                        nc.gpsimd.dma_start(out=x_t[:], in_=x_seq[roff:roff + P, :])
                        xot = p_pw.tile([P, C], BF16, tag="xot")
                        for cc in range(2):
                            pm = ps_pr.tile([P, CH], F32)
                            for hp in range(NC8):
                                nc.tensor.matmul(
                                    pm[:], yT_sb[:, hp, qt * P:(qt + 1) * P],
                                    wp_sb[:, hp, cc * CH:(cc + 1) * CH],
                                    start=(hp == 0), stop=(hp == NC8 - 1))
                            nc.vector.tensor_add(
                                out=xot[:, cc * CH:(cc + 1) * CH],
                                in0=pm[:], in1=x_t[:, cc * CH:(cc + 1) * CH])
                        nc.sync.dma_start(out=xo_dram[qt * P:(qt + 1) * P, :], in_=xot[:])
                        return xot

                    def ln2_unit(qt, xot):
                        h2_t = ln_common(xot[:], p_pw, "l2")
                        for co in range(NC8):
                            tp = ps_t2.tile([P, P], BF16)
                            nc.tensor.transpose(tp[:], h2_t[:, co * P:(co + 1) * P], id_sb[:])
                            copy_evict(h2T_sb[:, co, qt * P:(qt + 1) * P], tp[:])

                    xots = {}
                    for qt in range(NC8):
                        xots[qt] = proj_unit(qt)
                        if qt >= 1:
                            ln2_unit(qt - 1, xots.pop(qt - 1))
                    ln2_unit(NC8 - 1, xots.pop(NC8 - 1))
                _cm_yT.__exit__(None, None, None)

            # ================= P3: fc1 + fc2 =================
            with tc.tile_pool(name="p_a1", bufs=1) as p_a1, \
                 tc.tile_pool(name="p_w2", bufs=1) as p_w2:
                a1_sb = p_a1.tile([P, F // P, 2 * CH], BF16)
                w2_sb = p_w2.tile([P, F // P, C], BF16)
                for fo in range(4):
                    nc.gpsimd.dma_start(
                        out=w2_sb[:, fo * 8:(fo + 1) * 8, :],
                        in_=w2r[:, fo * 8:(fo + 1) * 8, :])
                with tc.tile_pool(name="p_w1", bufs=2) as p_w1, \
                     tc.tile_pool(name="ps_f1", bufs=4, space="PSUM") as ps_f1:
                    FQ = F // 4
                    for fh in range(4):
                        w1 = p_w1.tile([P, NC8, FQ], BF16, tag="w1")
                        nc.gpsimd.dma_start(
                            out=w1[:], in_=w1r[:, :, fh * FQ:(fh + 1) * FQ])
                        for ft in range(FQ // P):
                            fg = fh * (FQ // P) + ft
                            for qc in range(2):
                                pm = ps_f1.tile([P, CH], F32)
                                for ct in range(NC8):
                                    nc.tensor.matmul(
                                        pm[:], w1[:, ct, ft * P:(ft + 1) * P],
                                        h2T_sb[:, ct, qc * CH:(qc + 1) * CH],
                                        start=(ct == 0), stop=(ct == NC8 - 1))
                                if add_bfc1:
                                    nc.scalar.activation(
                                        a1_sb[:, fg, qc * CH:(qc + 1) * CH], pm[:],
                                        mybir.ActivationFunctionType.Relu,
                                        bias=bfc1_sb[:, fg:fg + 1])
                                elif fg % 2 == 0:
                                    nc.scalar.activation(
                                        a1_sb[:, fg, qc * CH:(qc + 1) * CH], pm[:],
                                        mybir.ActivationFunctionType.Relu)
                                else:
                                    nc.vector.tensor_scalar_max(
                                        out=a1_sb[:, fg, qc * CH:(qc + 1) * CH],
                                        in0=pm[:], scalar1=0.0)

                # ---- fc2: one 32-matmul accumulation chain per (qt, cc) ----
                with tc.tile_pool(name="p_os", bufs=2) as p_os, \
                     tc.tile_pool(name="ps_f2", bufs=3, space="PSUM") as ps_f2:
                    for qt in range(NC8):
                        xoq = p_os.tile([P, C], BF16, tag="xoq")
                        nc.sync.dma_start(out=xoq[:], in_=xo_dram[qt * P:(qt + 1) * P, :])
                        ost = p_os.tile([P, C], F32, tag="ost")
                        for cc in range(2):
                            pm = ps_f2.tile([P, CH], F32)
                            for ft in range(F // P):
                                nc.tensor.matmul(
                                    pm[:], a1_sb[:, ft, qt * P:(qt + 1) * P],
                                    w2_sb[:, ft, cc * CH:(cc + 1) * CH],
                                    start=(ft == 0), stop=(ft == F // P - 1))
                            nc.vector.tensor_add(
                                out=ost[:, cc * CH:(cc + 1) * CH],
                                in0=pm[:], in1=xoq[:, cc * CH:(cc + 1) * CH])
                        if add_bfc2:
                            nc.vector.tensor_add(out=ost[:], in0=ost[:], in1=bfc2_sb[:])
                        nc.sync.dma_start(out=out[qt * P:(qt + 1) * P, :], in_=ost[:])
            _cm_dram.__exit__(None, None, None)
            _cm_h2T.__exit__(None, None, None)
    nc.compile()
    return nc


def _perm(par):
    return [0, 1, 2, 3] if par == 0 else [1, 0, 3, 2]


def _masks(par):
    perm = _perm(par)
    real = np.empty(T, np.int64)
    for c in range(4):
        real[c * CH:(c + 1) * CH] = perm[c] * CH + np.arange(CH)
    p = np.arange(P)
    j = np.arange(CH)
    kt = np.arange(8)
    # maskA: ci0 (q = perm pos j), kt tiles 0..7
    kA = real[(kt[None, :, None] * P + p[:, None, None]) % T]
    qA = real[j][None, None, :]
    mA = (kA <= qA).astype(np.float32)
    # maskB: ci1 (q = perm pos 1536+j), kt tiles 8..15
    kB = real[((8 + kt[None, :, None]) * P + p[:, None, None]) % T]
    qB = real[3 * CH + j][None, None, :]
    mB = (kB <= qB).astype(np.float32)
    bf = lambda a: np.ascontiguousarray(a).astype(bfloat16)
    return bf(mA), bf(mB)


def build_in_maps(x, ln1_w, w_attn, w_proj, ln2_w, w_fc1, b_fc1, w_fc2, b_fc2):
    x = np.asarray(x, np.float32)
    B = x.shape[0]
    bf = lambda a: np.ascontiguousarray(np.asarray(a, np.float32)).astype(bfloat16)
    ln1 = np.asarray(ln1_w, np.float32)[:, None]
    ln2 = np.asarray(ln2_w, np.float32)[:, None]
    shared = dict(
        w_attn=bf(ln1 * np.asarray(w_attn, np.float32)),
        w_proj=bf(w_proj),
        w_fc1=bf(ln2 * np.asarray(w_fc1, np.float32)),
        w_fc2=bf(w_fc2),
        bfc1_col=np.ascontiguousarray(np.asarray(b_fc1, np.float32).reshape(F // P, P).T),
        bfc2_rep=np.tile(np.asarray(b_fc2, np.float32)[None, :], (P, 1)),
        ident=np.eye(P, dtype=np.float32).astype(bfloat16),
    )
    masks = [_masks(0), _masks(1)]
    in_maps = []
    for core in range(2 * B):
        s, par = core // 2, core % 2
        perm = _perm(par)
        xp = np.concatenate([x[s, c * CH:(c + 1) * CH] for c in perm], 0)
        mA, mB = masks[par]
        in_maps.append(dict(shared, x_seq=np.ascontiguousarray(xp),
                            maskA=mA, maskB=mB))
    return in_maps


def kernel(x, ln1_w, w_attn, w_proj, ln2_w, w_fc1, b_fc1, w_fc2, b_fc2):
    x = np.asarray(x, np.float32)
    B = x.shape[0]
    add_bfc1 = not np.allclose(b_fc1, 0.0)
    add_bfc2 = not np.allclose(b_fc2, 0.0)
    key = (add_bfc1, add_bfc2)
    if key not in _cache:
        _cache[key] = build_nc(*key)
    nc = _cache[key]

    in_maps = build_in_maps(x, ln1_w, w_attn, w_proj, ln2_w, w_fc1, b_fc1,
                            w_fc2, b_fc2)
    res = run_bass_kernel_spmd(nc, in_maps, list(range(2 * B)))
    out = np.empty_like(x)
    for core in range(2 * B):
        s, par = core // 2, core % 2
        perm = _perm(par)
        r = res.results[core]["out"]
        out[s, perm[0] * CH:(perm[0] + 1) * CH] = r[0:CH]
        out[s, perm[3] * CH:(perm[3] + 1) * CH] = r[CH:2 * CH]
    return out


# revision 12
# speedup vs baseline: 1.0174x; 1.0174x over previous
"""Trainium2 Bass kernel for a dense transformer block (B=4,T=2048,C=1024,H=16,F=4096).

Sharding (8 cores, no collectives): core 2s+p owns sequence s (p=parity).
Each core handles two 512-token q-chunks; the host PERMUTES the sequence
per-core so the q-chunks always sit at permuted positions [0:512] and
[1536:2048] (parity 0: chunks (0,3) with identity perm; parity 1: chunks
(1,2) with perm [1,0,3,2]).  That makes every operand offset uniform across
cores (single SPMD program); causality lives entirely in per-core mask data.

Attention computes scores pre-transposed (S^T[k,q] = K Q^T) with the two
heads of a 128-partition pair issued as back-to-back row-tiled matmuls
(tile_position (0,0)/(64,0)) so they run concurrently in the PE array.
exp is unnormalized; the denominator comes from a ones-column appended to V
and is partition-broadcast via a tiny fp32 matmul (no DRAM roundtrip).
Causal masks are multiplied in on the GPSIMD engine.  Dummy "tickle"
matmuls tied to exp outputs keep the PE HAM clock warm through the
scalar-bound attention phase.
"""
import sys, types
import numpy as np
import ml_dtypes

# --- make the NTFF profile hook importable (missing module in this image) ---
def _install_hooks():
    try:
        import antenv
        if "antenv.axon_hooks" not in sys.modules:
            m = types.ModuleType("antenv.axon_hooks")
            m._hook = None
            m.set_axon_ntff_profile_hook = lambda h: setattr(m, "_hook", h)
            m.get_axon_ntff_profile_hook = lambda: m._hook
            sys.modules["antenv.axon_hooks"] = m
            antenv.axon_hooks = m
    except Exception:
        pass
_install_hooks()

import concourse.bass as bass
import concourse.tile as tile
from concourse import mybir, bacc
from concourse.bass_utils import run_bass_kernel_spmd

BF16 = mybir.dt.bfloat16
F32 = mybir.dt.float32
bfloat16 = ml_dtypes.bfloat16

T, C, H, D, F = 2048, 1024, 16, 64, 4096
CH = 512            # q-chunk width
P = 128
NT = T // P         # 16 token tiles
NC8 = C // P        # 8 contraction tiles
NKT = (8, 16)       # kt-tile slots per (ci=0, ci=1)
QOFF = (0, 3 * CH)  # permuted-seq offsets of the two q-chunks

_cache = {}


def build_nc(add_bfc1, add_bfc2):
    nc = bacc.Bacc()
    x_seq = nc.declare_dram_parameter("x_seq", [T, C], F32, isOutput=False)
    w_attn = nc.declare_dram_parameter("w_attn", [C, 3 * C], BF16, isOutput=False)
    w_proj = nc.declare_dram_parameter("w_proj", [C, C], BF16, isOutput=False)
    w_fc1 = nc.declare_dram_parameter("w_fc1", [C, F], BF16, isOutput=False)
    w_fc2 = nc.declare_dram_parameter("w_fc2", [F, C], BF16, isOutput=False)
    bfc1 = nc.declare_dram_parameter("bfc1_col", [P, F // P], F32, isOutput=False)
    bfc2 = nc.declare_dram_parameter("bfc2_rep", [P, C], F32, isOutput=False)
    ident = nc.declare_dram_parameter("ident", [P, P], BF16, isOutput=False)
    maskA = nc.declare_dram_parameter("maskA", [P, 8, CH], BF16, isOutput=False)
    maskB = nc.declare_dram_parameter("maskB", [P, 8, CH], BF16, isOutput=False)
    out = nc.declare_dram_parameter("out", [2 * CH, C], F32, isOutput=True)

    wAr = w_attn.rearrange("(co p) f -> p co f", p=P)
    wPr = w_proj.rearrange("(co p) f -> p co f", p=P)
    w1r = w_fc1.rearrange("(co p) f -> p co f", p=P)
    w2r = w_fc2.rearrange("(fo p) c -> p fo c", p=P)

    ec = [0]  # evict-engine round robin

    with tile.TileContext(nc, pool_alloc_mode="queue") as tc:
        with tc.tile_pool(name="consts", bufs=1) as consts:
            id_sb = consts.tile([P, P], BF16)
            nc.sync.dma_start(out=id_sb[:], in_=ident[:])
            eps_sb = consts.tile([P, 1], F32)
            nc.vector.memset(eps_sb[:], 1e-5)
            ones_sb = consts.tile([P, D], F32)
            nc.vector.memset(ones_sb[:], 1.0)
            bfc1_sb = bfc2_sb = None
            if add_bfc1:
                bfc1_sb = consts.tile([P, F // P], F32)
                nc.sync.dma_start(out=bfc1_sb[:], in_=bfc1[:])
            if add_bfc2:
                bfc2_sb = consts.tile([P, C], F32)
                nc.sync.dma_start(out=bfc2_sb[:], in_=bfc2[:])

            def copy_evict(dst, src):
                if ec[0] % 3 == 0:
                    nc.vector.tensor_copy(out=dst, in_=src)
                else:
                    nc.scalar.copy(out=dst, in_=src)
                ec[0] += 1

            def ln_common(src_ap, pool, tagp):
                st = pool.tile([P, 2, 6], F32, tag=tagp + "st")
                xr = src_ap.rearrange("p (s d) -> p s d", s=2)
                for s in range(2):
                    nc.vector.bn_stats(out=st[:, s, :], in_=xr[:, s, :])
                mv = pool.tile([P, 2], F32, tag=tagp + "mv")
                nc.vector.bn_aggr(out=mv[:], in_=st[:])
                rstd = pool.tile([P, 1], F32, tag=tagp + "rs")
                nc.scalar.activation(out=rstd[:], in_=mv[:, 1:2],
                                     func=mybir.ActivationFunctionType.Sqrt,
                                     bias=eps_sb[:], scale=1.0)
                nc.vector.reciprocal(out=rstd[:], in_=rstd[:])
                h_t = pool.tile([P, C], BF16, tag=tagp + "ht")
                nc.vector.tensor_scalar(
                    out=h_t[:], in0=src_ap, scalar1=mv[:, 0:1],
                    scalar2=rstd[:], op0=mybir.AluOpType.subtract,
                    op1=mybir.AluOpType.mult)
                return h_t

            _cm_h2T = tc.tile_pool(name="p_h2T", bufs=1)
            p_h2T = _cm_h2T.__enter__()
            h2T_sb = p_h2T.tile([P, NC8, 2 * CH], BF16)
            _cm_dram = tc.tile_pool(name="p_dram", bufs=1, space="DRAM")
            p_dram = _cm_dram.__enter__()
            xo_dram = p_dram.tile([2 * CH, C], BF16)
            _cm_yT = tc.tile_pool(name="p_yT", bufs=1)
            p_yT = _cm_yT.__enter__()
            yT_sb = p_yT.tile([P, NC8, 2 * CH], BF16)
            _cm_kqv = tc.tile_pool(name="p_kqv", bufs=1)
            p_kqv = _cm_kqv.__enter__()
            if True:
                kT_sb = p_kqv.tile([P, NC8, T], BF16)       # [d(2 heads)][hp][t]
                qT_sb = p_kqv.tile([P, NC8, 2 * CH], BF16)
                v_sb = p_kqv.tile([P, NT, H, D + 1], BF16)  # ones col at [..,64]
                nc.vector.memset(v_sb[:, :, :, D:D + 1], 1.0)

                # ================= P1: ln1 + transpose + QKV =================
                with tc.tile_pool(name="p_wA", bufs=1) as p_wA, \
                     tc.tile_pool(name="p_hT", bufs=2) as p_hT, \
                     tc.tile_pool(name="p_ln", bufs=2) as p_ln, \
                     tc.tile_pool(name="ps_tr", bufs=2, space="PSUM") as ps_tr, \
                     tc.tile_pool(name="ps_mm", bufs=6, space="PSUM") as ps_mm:
                    wA_sb = p_wA.tile([P, NC8, 3 * C], BF16)
                    # K cols first (first use), then V, then Q; split so x-tile
                    # DMAs aren't stuck behind one 6MB transfer
                    for c0 in (C, 3 * C // 2, 2 * C, 5 * C // 2, 0, C // 2):
                        nc.gpsimd.dma_start(out=wA_sb[:, :, c0:c0 + C // 2],
                                            in_=wAr[:, :, c0:c0 + C // 2])
                    # hT is a rolling per-chunk transposed-activation buffer;
                    # the two q-chunks are exactly perm chunks 0 and 3.
                    hTcs = {}

                    def get_hTc(tcx):
                        if tcx not in hTcs:
                            hTcs[tcx] = p_hT.tile([P, NC8, CH], BF16, tag="hTc",
                                                  name=f"hTc{tcx}")
                        return hTcs[tcx]

                    def ln_unit(tt):
                        hTc = get_hTc(tt // 4)
                        t4 = tt % 4
                        x_t = p_ln.tile([P, C], F32, tag="xt")
                        nc.sync.dma_start(out=x_t[:], in_=x_seq[tt * P:(tt + 1) * P, :])
                        h1_t = ln_common(x_t[:], p_ln, "l1")
                        for co in range(NC8):
                            tp = ps_tr.tile([P, P], BF16)
                            nc.tensor.transpose(tp[:], h1_t[:, co * P:(co + 1) * P], id_sb[:])
                            copy_evict(hTc[:, co, t4 * P:(t4 + 1) * P], tp[:])

                    def k_unit(tcx, hp):
                        hTc = hTcs[tcx]
                        pm = ps_mm.tile([P, CH], F32)
                        for ct in range(NC8):
                            nc.tensor.matmul(
                                pm[:], wA_sb[:, ct, C + hp * P:C + (hp + 1) * P],
                                hTc[:, ct, :],
                                start=(ct == 0), stop=(ct == NC8 - 1))
                        copy_evict(kT_sb[:, hp, tcx * CH:(tcx + 1) * CH], pm[:])

                    def v_unit(tcx, tt4, fh):
                        hTc = hTcs[tcx]
                        tt = tcx * 4 + tt4
                        pm = ps_mm.tile([P, CH], F32)
                        for ct in range(NC8):
                            nc.tensor.matmul(
                                pm[:], hTc[:, ct, tt4 * P:(tt4 + 1) * P],
                                wA_sb[:, ct, 2 * C + fh * CH:2 * C + (fh + 1) * CH],
                                start=(ct == 0), stop=(ct == NC8 - 1))
                        nc.vector.tensor_copy(
                            out=v_sb[:, tt, fh * 8:(fh + 1) * 8, 0:D],
                            in_=pm.rearrange("p (h d) -> p h d", d=D))

                    def q_unit(ci, hp):
                        hTc = hTcs[0 if ci == 0 else 3]
                        pm = ps_mm.tile([P, CH], F32)
                        for ct in range(NC8):
                            nc.tensor.matmul(
                                pm[:], wA_sb[:, ct, hp * P:(hp + 1) * P],
                                hTc[:, ct, :],
                                start=(ct == 0), stop=(ct == NC8 - 1))
                        copy_evict(qT_sb[:, hp, ci * CH:(ci + 1) * CH], pm[:])

                    for tt in range(4):
                        ln_unit(tt)
                    for tcx in range(4):
                        if tcx == 0:
                            units = [lambda tt4=tt4, fh=fh: v_unit(tcx, tt4, fh)
                                     for tt4 in range(4) for fh in range(2)]
                            units += [lambda hp=hp: k_unit(tcx, hp) for hp in range(NC8)]
                            units += [lambda hp=hp: q_unit(0, hp) for hp in range(NC8)]
                        else:
                            units = [lambda hp=hp: k_unit(tcx, hp) for hp in range(NC8)]
                            units += [lambda tt4=tt4, fh=fh: v_unit(tcx, tt4, fh)
                                      for tt4 in range(4) for fh in range(2)]
                        nxt = list(range(4 * (tcx + 1), min(4 * (tcx + 2), NT)))
                        for i, u in enumerate(units):
                            u()
                            if i % 5 == 4 and nxt:
                                ln_unit(nxt.pop(0))
                        for tt in nxt:
                            ln_unit(tt)
                    for hp in range(NC8):
                        q_unit(1, hp)

                # ================= P2: attention =================
                with tc.tile_pool(name="p_mA", bufs=1) as p_mA, \
                     tc.tile_pool(name="p_mB", bufs=1) as p_mB, \
                     tc.tile_pool(name="p_pt", bufs=2) as p_pt, \
                     tc.tile_pool(name="p_aw", bufs=2) as p_aw, \
                     tc.tile_pool(name="ps_sc", bufs=2, space="PSUM") as ps_sc, \
                     tc.tile_pool(name="ps_av", bufs=2, space="PSUM") as ps_av, \
                     tc.tile_pool(name="ps_bc", bufs=1, space="PSUM") as ps_bc, \
                     tc.tile_pool(name="ps_tk", bufs=1, space="PSUM") as ps_tk:
                    mA_sb = p_mA.tile([P, 8, CH], BF16)
                    nc.sync.dma_start(out=mA_sb[:], in_=maskA[:])
                    mB_sb = p_mB.tile([P, 8, CH], BF16)
                    nc.sync.dma_start(out=mB_sb[:], in_=maskB[:])
                    tick = ps_tk.tile([1, D], F32)

                    pairs = [(ci, hp) for ci in range(2) for hp in range(NC8)]
                    state = {}

                    def tickle(prev_pt, g):
                        # tiny matmul reading a just-exp'd pt slice: keeps HAM warm
                        nc.tensor.matmul(tick[:], id_sb[:, 0:1],
                                         prev_pt[:, g % 8, 0, 0:D],
                                         start=True, stop=True)

                    def sxm_unit(pi):
                        ci, hp = pairs[pi]
                        n_kt = NKT[ci]
                        pt = p_pt.tile([P, NKT[1], 2, CH], BF16, tag="pt")
                        prev = state.get(pi - 1)
                        for g in range(n_kt):
                            sp = ps_sc.tile([P, 2, CH], F32)
                            for h2 in range(2):
                                nc.tensor.matmul(
                                    sp[:, h2, :],
                                    kT_sb[h2 * D:(h2 + 1) * D, hp, g * P:(g + 1) * P],
                                    qT_sb[h2 * D:(h2 + 1) * D, hp, ci * CH:(ci + 1) * CH],
                                    start=True, stop=True,
                                    tile_position=(h2 * D, 0))
                            if prev is not None and g % 2 == 0:
                                tickle(prev, g)
                            nc.scalar.activation(
                                pt[:, g, :, :], sp[:],
                                mybir.ActivationFunctionType.Exp, scale=0.125)
                        for h2 in range(2):
                            if ci == 0:
                                nc.vector.tensor_mul(out=pt[:, 0:8, h2, :],
                                                     in0=pt[:, 0:8, h2, :], in1=mA_sb[:])
                            else:
                                nc.vector.tensor_mul(out=pt[:, 8:16, h2, :],
                                                     in0=pt[:, 8:16, h2, :], in1=mB_sb[:])
                        state[pi] = pt

                    def av_unit(pi, h2):
                        ci, hp = pairs[pi]
                        n_kt = NKT[ci]
                        pt = state[pi]
                        ap = ps_av.tile([D + 1, CH], F32)
                        for kt in range(n_kt):
                            nc.tensor.matmul(
                                ap[:], v_sb[:, kt, 2 * hp + h2, :], pt[:, kt, h2, :],
                                start=(kt == 0), stop=(kt == n_kt - 1))
                        dnS = p_aw.tile([D + 1, CH], F32, tag="dn")
                        nc.vector.tensor_copy(out=dnS[D:D + 1, :], in_=ap[D:D + 1, :])
                        bc = ps_bc.tile([D, CH], F32)
                        nc.tensor.matmul(bc[:], ones_sb[D:D + 1, 0:D], dnS[D:D + 1, :],
                                         start=True, stop=True)
                        rc = p_aw.tile([D, CH], F32, tag="rc")
                        nc.vector.reciprocal_approx_fast(out=rc[:], in_=bc[:])
                        if h2 == 0:
                            nc.vector.tensor_mul(
                                out=yT_sb[0:D, hp, ci * CH:(ci + 1) * CH],
                                in0=ap[0:D, :], in1=rc[:])
                        else:
                            yt = p_aw.tile([D, CH], BF16, tag="ytmp")
                            nc.vector.tensor_mul(out=yt[:], in0=ap[0:D, :], in1=rc[:])
                            nc.sync.dma_start(
                                out=yT_sb[D:P, hp, ci * CH:(ci + 1) * CH], in_=yt[:])

                    # software pipeline: scores/exp/mask leads AV by one pair
                    for pi in range(len(pairs) + 1):
                        if pi < len(pairs):
                            sxm_unit(pi)
                        if pi >= 1:
                            av_unit(pi - 1, 0)
                            av_unit(pi - 1, 1)
                            state.pop(pi - 2, None)

                # ============ proj + residual + ln2 ============
                _cm_kqv.__exit__(None, None, None)
                with tc.tile_pool(name="p_wp", bufs=1) as p_wp, \
                     tc.tile_pool(name="p_pw", bufs=2) as p_pw, \
                     tc.tile_pool(name="ps_pr", bufs=3, space="PSUM") as ps_pr, \
                     tc.tile_pool(name="ps_t2", bufs=2, space="PSUM") as ps_t2:
                    wp_sb = p_wp.tile([P, NC8, C], BF16)
                    nc.sync.dma_start(out=wp_sb[:], in_=wPr[:])

                    def proj_unit(qt):
                        x_t = p_pw.tile([P, C], F32, tag="xq")
                        roff = QOFF[qt // 4] + (qt % 4) * P
                        nc.gpsimd.dma_start(out=x_t[:], in_=x_seq[roff:roff + P, :])
                        xot = p_pw.tile([P, C], BF16, tag="xot")
                        for cc in range(2):
                            pm = ps_pr.tile([P, CH], F32)
                            for hp in range(NC8):
                                nc.tensor.matmul(
                                    pm[:], yT_sb[:, hp, qt * P:(qt + 1) * P],
                                    wp_sb[:, hp, cc * CH:(cc + 1) * CH],
                                    start=(hp == 0), stop=(hp == NC8 - 1))
                            nc.vector.tensor_add(
                                out=xot[:, cc * CH:(cc + 1) * CH],
                                in0=pm[:], in1=x_t[:, cc * CH:(cc + 1) * CH])
                        nc.sync.dma_start(out=xo_dram[qt * P:(qt + 1) * P, :], in_=xot[:])
                        return xot

                    def ln2_unit(qt, xot):
                        h2_t = ln_common(xot[:], p_pw, "l2")
                        for co in range(NC8):
                            tp = ps_t2.tile([P, P], BF16)
                            nc.tensor.transpose(tp[:], h2_t[:, co * P:(co + 1) * P], id_sb[:])
                            copy_evict(h2T_sb[:, co, qt * P:(qt + 1) * P], tp[:])

                    xots = {}
                    for qt in range(NC8):
                        xots[qt] = proj_unit(qt)
                        if qt >= 1:
                            ln2_unit(qt - 1, xots.pop(qt - 1))
                    ln2_unit(NC8 - 1, xots.pop(NC8 - 1))
                _cm_yT.__exit__(None, None, None)

            # ================= P3: fc1 + fc2 =================
            with tc.tile_pool(name="p_a1", bufs=1) as p_a1, \
                 tc.tile_pool(name="p_w2", bufs=1) as p_w2:
                a1_sb = p_a1.tile([P, F // P, 2 * CH], BF16)
                w2_sb = p_w2.tile([P, F // P, C], BF16)
                for fo in range(4):
                    nc.gpsimd.dma_start(
                        out=w2_sb[:, fo * 8:(fo + 1) * 8, :],
                        in_=w2r[:, fo * 8:(fo + 1) * 8, :])
                with tc.tile_pool(name="p_w1", bufs=2) as p_w1, \
                     tc.tile_pool(name="ps_f1", bufs=5, space="PSUM") as ps_f1:
                    FQ = F // 4
                    for fh in range(4):
                        w1 = p_w1.tile([P, NC8, FQ], BF16, tag="w1")
                        nc.gpsimd.dma_start(
                            out=w1[:], in_=w1r[:, :, fh * FQ:(fh + 1) * FQ])
                        for ft in range(FQ // P):
                            fg = fh * (FQ // P) + ft
                            for qc in range(2):
                                pm = ps_f1.tile([P, CH], F32)
                                for ct in range(NC8):
                                    nc.tensor.matmul(
                                        pm[:], w1[:, ct, ft * P:(ft + 1) * P],
                                        h2T_sb[:, ct, qc * CH:(qc + 1) * CH],
                                        start=(ct == 0), stop=(ct == NC8 - 1))
                                if add_bfc1:
                                    nc.scalar.activation(
                                        a1_sb[:, fg, qc * CH:(qc + 1) * CH], pm[:],
                                        mybir.ActivationFunctionType.Relu,
                                        bias=bfc1_sb[:, fg:fg + 1])
                                elif fg % 2 == 0:
                                    nc.scalar.activation(
                                        a1_sb[:, fg, qc * CH:(qc + 1) * CH], pm[:],
                                        mybir.ActivationFunctionType.Relu)
                                else:
                                    nc.vector.tensor_scalar_max(
                                        out=a1_sb[:, fg, qc * CH:(qc + 1) * CH],
                                        in0=pm[:], scalar1=0.0)

                # ---- fc2: one 32-matmul accumulation chain per (qt, cc) ----
                with tc.tile_pool(name="p_os", bufs=2) as p_os, \
                     tc.tile_pool(name="ps_f2", bufs=4, space="PSUM") as ps_f2:
                    for qt in range(NC8):
                        xoq = p_os.tile([P, C], BF16, tag="xoq")
                        nc.sync.dma_start(out=xoq[:], in_=xo_dram[qt * P:(qt + 1) * P, :])
                        ost = p_os.tile([P, C], F32, tag="ost")
                        for cc in range(2):
                            pm = ps_f2.tile([P, CH], F32)
                            for ft in range(F // P):
                                nc.tensor.matmul(
                                    pm[:], a1_sb[:, ft, qt * P:(qt + 1) * P],
                                    w2_sb[:, ft, cc * CH:(cc + 1) * CH],
                                    start=(ft == 0), stop=(ft == F // P - 1))
                            nc.vector.tensor_add(
                                out=ost[:, cc * CH:(cc + 1) * CH],
                                in0=pm[:], in1=xoq[:, cc * CH:(cc + 1) * CH])
                            if add_bfc2:
                                nc.vector.tensor_add(
                                    out=ost[:, cc * CH:(cc + 1) * CH],
                                    in0=ost[:, cc * CH:(cc + 1) * CH],
                                    in1=bfc2_sb[:, cc * CH:(cc + 1) * CH])
                            nc.sync.dma_start(
                                out=out[qt * P:(qt + 1) * P, cc * CH:(cc + 1) * CH],
                                in_=ost[:, cc * CH:(cc + 1) * CH])
            _cm_dram.__exit__(None, None, None)
            _cm_h2T.__exit__(None, None, None)
    nc.compile()
    return nc


def _perm(par):
    return [0, 1, 2, 3] if par == 0 else [1, 0, 3, 2]


def _masks(par):
    perm = _perm(par)
    real = np.empty(T, np.int64)
    for c in range(4):
        real[c * CH:(c + 1) * CH] = perm[c] * CH + np.arange(CH)
    p = np.arange(P)
    j = np.arange(CH)
    kt = np.arange(8)
    # maskA: ci0 (q = perm pos j), kt tiles 0..7
    kA = real[(kt[None, :, None] * P + p[:, None, None]) % T]
    qA = real[j][None, None, :]
    mA = (kA <= qA).astype(np.float32)
    # maskB: ci1 (q = perm pos 1536+j), kt tiles 8..15
    kB = real[((8 + kt[None, :, None]) * P + p[:, None, None]) % T]
    qB = real[3 * CH + j][None, None, :]
    mB = (kB <= qB).astype(np.float32)
    bf = lambda a: np.ascontiguousarray(a).astype(bfloat16)
    return bf(mA), bf(mB)


def build_in_maps(x, ln1_w, w_attn, w_proj, ln2_w, w_fc1, b_fc1, w_fc2, b_fc2):
    x = np.asarray(x, np.float32)
    B = x.shape[0]
    bf = lambda a: np.ascontiguousarray(np.asarray(a, np.float32)).astype(bfloat16)
    ln1 = np.asarray(ln1_w, np.float32)[:, None]
    ln2 = np.asarray(ln2_w, np.float32)[:, None]
    shared = dict(
        w_attn=bf(ln1 * np.asarray(w_attn, np.float32)),
        w_proj=bf(w_proj),
        w_fc1=bf(ln2 * np.asarray(w_fc1, np.float32)),
        w_fc2=bf(w_fc2),
        bfc1_col=np.ascontiguousarray(np.asarray(b_fc1, np.float32).reshape(F // P, P).T),
        bfc2_rep=np.tile(np.asarray(b_fc2, np.float32)[None, :], (P, 1)),
        ident=np.eye(P, dtype=np.float32).astype(bfloat16),
    )
    masks = [_masks(0), _masks(1)]
    in_maps = []
    for core in range(2 * B):
        s, par = core // 2, core % 2
        perm = _perm(par)
        xp = np.concatenate([x[s, c * CH:(c + 1) * CH] for c in perm], 0)
        mA, mB = masks[par]
        in_maps.append(dict(shared, x_seq=np.ascontiguousarray(xp),
                            maskA=mA, maskB=mB))
    return in_maps


def kernel(x, ln1_w, w_attn, w_proj, ln2_w, w_fc1, b_fc1, w_fc2, b_fc2):
    x = np.asarray(x, np.float32)
    B = x.shape[0]
    add_bfc1 = not np.allclose(b_fc1, 0.0)
    add_bfc2 = not np.allclose(b_fc2, 0.0)
    key = (add_bfc1, add_bfc2)
    if key not in _cache:
        _cache[key] = build_nc(*key)
    nc = _cache[key]

    in_maps = build_in_maps(x, ln1_w, w_attn, w_proj, ln2_w, w_fc1, b_fc1,
                            w_fc2, b_fc2)
    res = run_bass_kernel_spmd(nc, in_maps, list(range(2 * B)))
    out = np.empty_like(x)
    for core in range(2 * B):
        s, par = core // 2, core % 2
        perm = _perm(par)
        r = res.results[core]["out"]
        out[s, perm[0] * CH:(perm[0] + 1) * CH] = r[0:CH]
        out[s, perm[3] * CH:(perm[3] + 1) * CH] = r[CH:2 * CH]
    return out


# revision 13
# speedup vs baseline: 1.0187x; 1.0014x over previous
"""Trainium2 Bass kernel for a dense transformer block (B=4,T=2048,C=1024,H=16,F=4096).

Sharding (8 cores, no collectives): core 2s+p owns sequence s (p=parity).
Each core handles two 512-token q-chunks; the host PERMUTES the sequence
per-core so the q-chunks always sit at permuted positions [0:512] and
[1536:2048] (parity 0: chunks (0,3) with identity perm; parity 1: chunks
(1,2) with perm [1,0,3,2]).  That makes every operand offset uniform across
cores (single SPMD program); causality lives entirely in per-core mask data.

Attention computes scores pre-transposed (S^T[k,q] = K Q^T) with the two
heads of a 128-partition pair issued as back-to-back row-tiled matmuls
(tile_position (0,0)/(64,0)) so they run concurrently in the PE array.
exp is unnormalized; the denominator comes from a ones-column appended to V
and is partition-broadcast via a tiny fp32 matmul (no DRAM roundtrip).
Causal masks are multiplied in on the GPSIMD engine.  Dummy "tickle"
matmuls tied to exp outputs keep the PE HAM clock warm through the
scalar-bound attention phase.
"""
import sys, types
import numpy as np
import ml_dtypes

# --- make the NTFF profile hook importable (missing module in this image) ---
def _install_hooks():
    try:
        import antenv
        if "antenv.axon_hooks" not in sys.modules:
            m = types.ModuleType("antenv.axon_hooks")
            m._hook = None
            m.set_axon_ntff_profile_hook = lambda h: setattr(m, "_hook", h)
            m.get_axon_ntff_profile_hook = lambda: m._hook
            sys.modules["antenv.axon_hooks"] = m
            antenv.axon_hooks = m
    except Exception:
        pass
_install_hooks()

import concourse.bass as bass
import concourse.tile as tile
from concourse import mybir, bacc
from concourse.bass_utils import run_bass_kernel_spmd

BF16 = mybir.dt.bfloat16
F32 = mybir.dt.float32
bfloat16 = ml_dtypes.bfloat16

T, C, H, D, F = 2048, 1024, 16, 64, 4096
CH = 512            # q-chunk width
P = 128
NT = T // P         # 16 token tiles
NC8 = C // P        # 8 contraction tiles
NKT = (8, 16)       # kt-tile slots per (ci=0, ci=1)
QOFF = (0, 3 * CH)  # permuted-seq offsets of the two q-chunks

_cache = {}


def build_nc(add_bfc1, add_bfc2):
    nc = bacc.Bacc()
    x_seq = nc.declare_dram_parameter("x_seq", [T, C], F32, isOutput=False)
    w_attn = nc.declare_dram_parameter("w_attn", [C, 3 * C], BF16, isOutput=False)
    w_proj = nc.declare_dram_parameter("w_proj", [C, C], BF16, isOutput=False)
    w_fc1 = nc.declare_dram_parameter("w_fc1", [C, F], BF16, isOutput=False)
    w_fc2 = nc.declare_dram_parameter("w_fc2", [F, C], BF16, isOutput=False)
    bfc1 = nc.declare_dram_parameter("bfc1_col", [P, F // P], F32, isOutput=False)
    bfc2 = nc.declare_dram_parameter("bfc2_rep", [P, C], F32, isOutput=False)
    ident = nc.declare_dram_parameter("ident", [P, P], BF16, isOutput=False)
    maskA = nc.declare_dram_parameter("maskA", [P, 8, CH], BF16, isOutput=False)
    maskB = nc.declare_dram_parameter("maskB", [P, 8, CH], BF16, isOutput=False)
    out = nc.declare_dram_parameter("out", [2 * CH, C], F32, isOutput=True)

    wAr = w_attn.rearrange("(co p) f -> p co f", p=P)
    wPr = w_proj.rearrange("(co p) f -> p co f", p=P)
    w1r = w_fc1.rearrange("(co p) f -> p co f", p=P)
    w2r = w_fc2.rearrange("(fo p) c -> p fo c", p=P)

    ec = [0]  # evict-engine round robin

    with tile.TileContext(nc, pool_alloc_mode="queue") as tc:
        with tc.tile_pool(name="consts", bufs=1) as consts:
            id_sb = consts.tile([P, P], BF16)
            nc.sync.dma_start(out=id_sb[:], in_=ident[:])
            eps_sb = consts.tile([P, 1], F32)
            nc.vector.memset(eps_sb[:], 1e-5)
            ones_sb = consts.tile([P, D], F32)
            nc.vector.memset(ones_sb[:], 1.0)
            bfc1_sb = bfc2_sb = None
            if add_bfc1:
                bfc1_sb = consts.tile([P, F // P], F32)
                nc.sync.dma_start(out=bfc1_sb[:], in_=bfc1[:])
            if add_bfc2:
                bfc2_sb = consts.tile([P, C], F32)
                nc.sync.dma_start(out=bfc2_sb[:], in_=bfc2[:])

            def copy_evict(dst, src):
                if ec[0] % 3 == 0:
                    nc.vector.tensor_copy(out=dst, in_=src)
                else:
                    nc.scalar.copy(out=dst, in_=src)
                ec[0] += 1

            def ln_common(src_ap, pool, tagp):
                st = pool.tile([P, 2, 6], F32, tag=tagp + "st")
                xr = src_ap.rearrange("p (s d) -> p s d", s=2)
                for s in range(2):
                    nc.vector.bn_stats(out=st[:, s, :], in_=xr[:, s, :])
                mv = pool.tile([P, 2], F32, tag=tagp + "mv")
                nc.vector.bn_aggr(out=mv[:], in_=st[:])
                rstd = pool.tile([P, 1], F32, tag=tagp + "rs")
                nc.scalar.activation(out=rstd[:], in_=mv[:, 1:2],
                                     func=mybir.ActivationFunctionType.Sqrt,
                                     bias=eps_sb[:], scale=1.0)
                nc.vector.reciprocal(out=rstd[:], in_=rstd[:])
                h_t = pool.tile([P, C], BF16, tag=tagp + "ht")
                nc.vector.tensor_scalar(
                    out=h_t[:], in0=src_ap, scalar1=mv[:, 0:1],
                    scalar2=rstd[:], op0=mybir.AluOpType.subtract,
                    op1=mybir.AluOpType.mult)
                return h_t

            _cm_h2T = tc.tile_pool(name="p_h2T", bufs=1)
            p_h2T = _cm_h2T.__enter__()
            h2T_sb = p_h2T.tile([P, NC8, 2 * CH], BF16)
            _cm_dram = tc.tile_pool(name="p_dram", bufs=1, space="DRAM")
            p_dram = _cm_dram.__enter__()
            xo_dram = p_dram.tile([2 * CH, C], BF16)
            _cm_yT = tc.tile_pool(name="p_yT", bufs=1)
            p_yT = _cm_yT.__enter__()
            yT_sb = p_yT.tile([P, NC8, 2 * CH], BF16)
            _cm_kqv = tc.tile_pool(name="p_kqv", bufs=1)
            p_kqv = _cm_kqv.__enter__()
            if True:
                kT_sb = p_kqv.tile([P, NC8, T], BF16)       # [d(2 heads)][hp][t]
                qT_sb = p_kqv.tile([P, NC8, 2 * CH], BF16)
                v_sb = p_kqv.tile([P, NT, H, D + 1], BF16)  # ones col at [..,64]
                nc.vector.memset(v_sb[:, :, :, D:D + 1], 1.0)

                # ================= P1: ln1 + transpose + QKV =================
                with tc.tile_pool(name="p_wA", bufs=1) as p_wA, \
                     tc.tile_pool(name="p_hT", bufs=2) as p_hT, \
                     tc.tile_pool(name="p_ln", bufs=2) as p_ln, \
                     tc.tile_pool(name="ps_tr", bufs=2, space="PSUM") as ps_tr, \
                     tc.tile_pool(name="ps_mm", bufs=6, space="PSUM") as ps_mm:
                    wA_sb = p_wA.tile([P, NC8, 3 * C], BF16)
                    # K cols first (first use), then V, then Q; split so x-tile
                    # DMAs aren't stuck behind one 6MB transfer
                    for c0 in (C, 3 * C // 2, 2 * C, 5 * C // 2, 0, C // 2):
                        nc.gpsimd.dma_start(out=wA_sb[:, :, c0:c0 + C // 2],
                                            in_=wAr[:, :, c0:c0 + C // 2])
                    # hT is a rolling per-chunk transposed-activation buffer;
                    # the two q-chunks are exactly perm chunks 0 and 3.
                    hTcs = {}

                    def get_hTc(tcx):
                        if tcx not in hTcs:
                            hTcs[tcx] = p_hT.tile([P, NC8, CH], BF16, tag="hTc",
                                                  name=f"hTc{tcx}")
                        return hTcs[tcx]

                    def ln_unit(tt):
                        hTc = get_hTc(tt // 4)
                        t4 = tt % 4
                        x_t = p_ln.tile([P, C], F32, tag="xt")
                        nc.sync.dma_start(out=x_t[:], in_=x_seq[tt * P:(tt + 1) * P, :])
                        h1_t = ln_common(x_t[:], p_ln, "l1")
                        for co in range(NC8):
                            tp = ps_tr.tile([P, P], BF16)
                            nc.tensor.transpose(tp[:], h1_t[:, co * P:(co + 1) * P], id_sb[:])
                            copy_evict(hTc[:, co, t4 * P:(t4 + 1) * P], tp[:])

                    def k_unit(tcx, hp):
                        hTc = hTcs[tcx]
                        pm = ps_mm.tile([P, CH], F32)
                        for ct in range(NC8):
                            nc.tensor.matmul(
                                pm[:], wA_sb[:, ct, C + hp * P:C + (hp + 1) * P],
                                hTc[:, ct, :],
                                start=(ct == 0), stop=(ct == NC8 - 1))
                        copy_evict(kT_sb[:, hp, tcx * CH:(tcx + 1) * CH], pm[:])

                    def v_unit(tcx, tt4, fh):
                        hTc = hTcs[tcx]
                        tt = tcx * 4 + tt4
                        pm = ps_mm.tile([P, CH], F32)
                        for ct in range(NC8):
                            nc.tensor.matmul(
                                pm[:], hTc[:, ct, tt4 * P:(tt4 + 1) * P],
                                wA_sb[:, ct, 2 * C + fh * CH:2 * C + (fh + 1) * CH],
                                start=(ct == 0), stop=(ct == NC8 - 1))
                        nc.vector.tensor_copy(
                            out=v_sb[:, tt, fh * 8:(fh + 1) * 8, 0:D],
                            in_=pm.rearrange("p (h d) -> p h d", d=D))

                    def q_unit(ci, hp):
                        hTc = hTcs[0 if ci == 0 else 3]
                        pm = ps_mm.tile([P, CH], F32)
                        for ct in range(NC8):
                            nc.tensor.matmul(
                                pm[:], wA_sb[:, ct, hp * P:(hp + 1) * P],
                                hTc[:, ct, :],
                                start=(ct == 0), stop=(ct == NC8 - 1))
                        copy_evict(qT_sb[:, hp, ci * CH:(ci + 1) * CH], pm[:])

                    for tt in range(4):
                        ln_unit(tt)
                    for tcx in range(4):
                        if tcx == 0:
                            units = [lambda tt4=tt4, fh=fh: v_unit(tcx, tt4, fh)
                                     for tt4 in range(4) for fh in range(2)]
                            units += [lambda hp=hp: k_unit(tcx, hp) for hp in range(NC8)]
                            units += [lambda hp=hp: q_unit(0, hp) for hp in range(NC8)]
                        else:
                            units = [lambda hp=hp: k_unit(tcx, hp) for hp in range(NC8)]
                            units += [lambda tt4=tt4, fh=fh: v_unit(tcx, tt4, fh)
                                      for tt4 in range(4) for fh in range(2)]
                        nxt = list(range(4 * (tcx + 1), min(4 * (tcx + 2), NT)))
                        for i, u in enumerate(units):
                            u()
                            if i % 5 == 4 and nxt:
                                ln_unit(nxt.pop(0))
                        for tt in nxt:
                            ln_unit(tt)
                    for hp in range(NC8):
                        q_unit(1, hp)

                # ================= P2: attention =================
                with tc.tile_pool(name="p_mA", bufs=1) as p_mA, \
                     tc.tile_pool(name="p_mB", bufs=1) as p_mB, \
                     tc.tile_pool(name="p_pt", bufs=2) as p_pt, \
                     tc.tile_pool(name="p_aw", bufs=2) as p_aw, \
                     tc.tile_pool(name="ps_sc", bufs=2, space="PSUM") as ps_sc, \
                     tc.tile_pool(name="ps_av", bufs=2, space="PSUM") as ps_av, \
                     tc.tile_pool(name="ps_bc", bufs=1, space="PSUM") as ps_bc, \
                     tc.tile_pool(name="ps_tk", bufs=1, space="PSUM") as ps_tk:
                    mA_sb = p_mA.tile([P, 8, CH], BF16)
                    nc.sync.dma_start(out=mA_sb[:], in_=maskA[:])
                    mB_sb = p_mB.tile([P, 8, CH], BF16)
                    nc.sync.dma_start(out=mB_sb[:], in_=maskB[:])
                    tick = ps_tk.tile([1, D], F32)

                    pairs = [(ci, hp) for ci in range(2) for hp in range(NC8)]
                    state = {}

                    def tickle(prev_pt, g):
                        # tiny matmul reading a just-exp'd pt slice: keeps HAM warm
                        nc.tensor.matmul(tick[:], id_sb[:, 0:1],
                                         prev_pt[:, g % 8, 0, 0:D],
                                         start=True, stop=True)

                    def sxm_unit(pi):
                        ci, hp = pairs[pi]
                        n_kt = NKT[ci]
                        pt = p_pt.tile([P, NKT[1], 2, CH], BF16, tag="pt")
                        prev = state.get(pi - 1)
                        for g in range(n_kt):
                            sp = ps_sc.tile([P, 2, CH], F32)
                            for h2 in range(2):
                                nc.tensor.matmul(
                                    sp[:, h2, :],
                                    kT_sb[h2 * D:(h2 + 1) * D, hp, g * P:(g + 1) * P],
                                    qT_sb[h2 * D:(h2 + 1) * D, hp, ci * CH:(ci + 1) * CH],
                                    start=True, stop=True,
                                    tile_position=(h2 * D, 0))
                            if prev is not None and g % 2 == 0:
                                tickle(prev, g)
                            nc.scalar.activation(
                                pt[:, g, :, :], sp[:],
                                mybir.ActivationFunctionType.Exp, scale=0.125)
                        for h2 in range(2):
                            if ci == 0:
                                nc.vector.tensor_mul(out=pt[:, 0:8, h2, :],
                                                     in0=pt[:, 0:8, h2, :], in1=mA_sb[:])
                            else:
                                nc.vector.tensor_mul(out=pt[:, 8:16, h2, :],
                                                     in0=pt[:, 8:16, h2, :], in1=mB_sb[:])
                        state[pi] = pt

                    def av_unit(pi, h2):
                        ci, hp = pairs[pi]
                        n_kt = NKT[ci]
                        pt = state[pi]
                        ap = ps_av.tile([D + 1, CH], F32)
                        for kt in range(n_kt):
                            nc.tensor.matmul(
                                ap[:], v_sb[:, kt, 2 * hp + h2, :], pt[:, kt, h2, :],
                                start=(kt == 0), stop=(kt == n_kt - 1))
                        dnS = p_aw.tile([D + 1, CH], F32, tag="dn")
                        nc.vector.tensor_copy(out=dnS[D:D + 1, :], in_=ap[D:D + 1, :])
                        bc = ps_bc.tile([D, CH], F32)
                        nc.tensor.matmul(bc[:], ones_sb[D:D + 1, 0:D], dnS[D:D + 1, :],
                                         start=True, stop=True)
                        rc = p_aw.tile([D, CH], F32, tag="rc")
                        nc.vector.reciprocal_approx_fast(out=rc[:], in_=bc[:])
                        if h2 == 0:
                            nc.vector.tensor_mul(
                                out=yT_sb[0:D, hp, ci * CH:(ci + 1) * CH],
                                in0=ap[0:D, :], in1=rc[:])
                        else:
                            yt = p_aw.tile([D, CH], BF16, tag="ytmp")
                            nc.vector.tensor_mul(out=yt[:], in0=ap[0:D, :], in1=rc[:])
                            nc.sync.dma_start(
                                out=yT_sb[D:P, hp, ci * CH:(ci + 1) * CH], in_=yt[:])

                    # software pipeline: scores/exp/mask leads AV by one pair
                    for pi in range(len(pairs) + 1):
                        if pi < len(pairs):
                            sxm_unit(pi)
                        if pi >= 1:
                            av_unit(pi - 1, 0)
                            av_unit(pi - 1, 1)
                            state.pop(pi - 2, None)

                # ============ proj + residual + ln2 ============
                _cm_kqv.__exit__(None, None, None)
                with tc.tile_pool(name="p_wp", bufs=1) as p_wp, \
                     tc.tile_pool(name="p_pw", bufs=3) as p_pw, \
                     tc.tile_pool(name="ps_pr", bufs=3, space="PSUM") as ps_pr, \
                     tc.tile_pool(name="ps_t2", bufs=2, space="PSUM") as ps_t2:
                    wp_sb = p_wp.tile([P, NC8, C], BF16)
                    nc.sync.dma_start(out=wp_sb[:], in_=wPr[:])

                    def proj_unit(qt):
                        x_t = p_pw.tile([P, C], F32, tag="xq")
                        roff = QOFF[qt // 4] + (qt % 4) * P
                        nc.gpsimd.dma_start(out=x_t[:], in_=x_seq[roff:roff + P, :])
                        xot = p_pw.tile([P, C], BF16, tag="xot")
                        for cc in range(2):
                            pm = ps_pr.tile([P, CH], F32)
                            for hp in range(NC8):
                                nc.tensor.matmul(
                                    pm[:], yT_sb[:, hp, qt * P:(qt + 1) * P],
                                    wp_sb[:, hp, cc * CH:(cc + 1) * CH],
                                    start=(hp == 0), stop=(hp == NC8 - 1))
                            nc.vector.tensor_add(
                                out=xot[:, cc * CH:(cc + 1) * CH],
                                in0=pm[:], in1=x_t[:, cc * CH:(cc + 1) * CH])
                        nc.sync.dma_start(out=xo_dram[qt * P:(qt + 1) * P, :], in_=xot[:])
                        return xot

                    def ln2_unit(qt, xot):
                        h2_t = ln_common(xot[:], p_pw, "l2")
                        for co in range(NC8):
                            tp = ps_t2.tile([P, P], BF16)
                            nc.tensor.transpose(tp[:], h2_t[:, co * P:(co + 1) * P], id_sb[:])
                            copy_evict(h2T_sb[:, co, qt * P:(qt + 1) * P], tp[:])

                    xots = {}
                    for qt in range(NC8):
                        xots[qt] = proj_unit(qt)
                        if qt >= 2:
                            ln2_unit(qt - 2, xots.pop(qt - 2))
                    ln2_unit(NC8 - 2, xots.pop(NC8 - 2))
                    ln2_unit(NC8 - 1, xots.pop(NC8 - 1))
                _cm_yT.__exit__(None, None, None)

            # ================= P3: fc1 + fc2 =================
            with tc.tile_pool(name="p_a1", bufs=1) as p_a1, \
                 tc.tile_pool(name="p_w2", bufs=1) as p_w2:
                a1_sb = p_a1.tile([P, F // P, 2 * CH], BF16)
                w2_sb = p_w2.tile([P, F // P, C], BF16)
                for fo in range(4):
                    nc.gpsimd.dma_start(
                        out=w2_sb[:, fo * 8:(fo + 1) * 8, :],
                        in_=w2r[:, fo * 8:(fo + 1) * 8, :])
                with tc.tile_pool(name="p_w1", bufs=2) as p_w1, \
                     tc.tile_pool(name="ps_f1", bufs=5, space="PSUM") as ps_f1:
                    FQ = F // 4
                    for fh in range(4):
                        w1 = p_w1.tile([P, NC8, FQ], BF16, tag="w1")
                        nc.gpsimd.dma_start(
                            out=w1[:], in_=w1r[:, :, fh * FQ:(fh + 1) * FQ])
                        for ft in range(FQ // P):
                            fg = fh * (FQ // P) + ft
                            for qc in range(2):
                                pm = ps_f1.tile([P, CH], F32)
                                for ct in range(NC8):
                                    nc.tensor.matmul(
                                        pm[:], w1[:, ct, ft * P:(ft + 1) * P],
                                        h2T_sb[:, ct, qc * CH:(qc + 1) * CH],
                                        start=(ct == 0), stop=(ct == NC8 - 1))
                                if add_bfc1:
                                    nc.scalar.activation(
                                        a1_sb[:, fg, qc * CH:(qc + 1) * CH], pm[:],
                                        mybir.ActivationFunctionType.Relu,
                                        bias=bfc1_sb[:, fg:fg + 1])
                                elif fg % 2 == 0:
                                    nc.scalar.activation(
                                        a1_sb[:, fg, qc * CH:(qc + 1) * CH], pm[:],
                                        mybir.ActivationFunctionType.Relu)
                                else:
                                    nc.vector.tensor_scalar_max(
                                        out=a1_sb[:, fg, qc * CH:(qc + 1) * CH],
                                        in0=pm[:], scalar1=0.0)

                # ---- fc2: one 32-matmul accumulation chain per (qt, cc) ----
                with tc.tile_pool(name="p_os", bufs=2) as p_os, \
                     tc.tile_pool(name="ps_f2", bufs=4, space="PSUM") as ps_f2:
                    for qt in range(NC8):
                        xoq = p_os.tile([P, C], BF16, tag="xoq")
                        nc.sync.dma_start(out=xoq[:], in_=xo_dram[qt * P:(qt + 1) * P, :])
                        ost = p_os.tile([P, C], F32, tag="ost")
                        for cc in range(2):
                            pm = ps_f2.tile([P, CH], F32)
                            for ft in range(F // P):
                                nc.tensor.matmul(
                                    pm[:], a1_sb[:, ft, qt * P:(qt + 1) * P],
                                    w2_sb[:, ft, cc * CH:(cc + 1) * CH],
                                    start=(ft == 0), stop=(ft == F // P - 1))
                            nc.vector.tensor_add(
                                out=ost[:, cc * CH:(cc + 1) * CH],
                                in0=pm[:], in1=xoq[:, cc * CH:(cc + 1) * CH])
                            if add_bfc2:
                                nc.vector.tensor_add(
                                    out=ost[:, cc * CH:(cc + 1) * CH],
                                    in0=ost[:, cc * CH:(cc + 1) * CH],
                                    in1=bfc2_sb[:, cc * CH:(cc + 1) * CH])
                            nc.sync.dma_start(
                                out=out[qt * P:(qt + 1) * P, cc * CH:(cc + 1) * CH],
                                in_=ost[:, cc * CH:(cc + 1) * CH])
            _cm_dram.__exit__(None, None, None)
            _cm_h2T.__exit__(None, None, None)
    nc.compile()
    return nc


def _perm(par):
    return [0, 1, 2, 3] if par == 0 else [1, 0, 3, 2]


def _masks(par):
    perm = _perm(par)
    real = np.empty(T, np.int64)
    for c in range(4):
        real[c * CH:(c + 1) * CH] = perm[c] * CH + np.arange(CH)
    p = np.arange(P)
    j = np.arange(CH)
    kt = np.arange(8)
    # maskA: ci0 (q = perm pos j), kt tiles 0..7
    kA = real[(kt[None, :, None] * P + p[:, None, None]) % T]
    qA = real[j][None, None, :]
    mA = (kA <= qA).astype(np.float32)
    # maskB: ci1 (q = perm pos 1536+j), kt tiles 8..15
    kB = real[((8 + kt[None, :, None]) * P + p[:, None, None]) % T]
    qB = real[3 * CH + j][None, None, :]
    mB = (kB <= qB).astype(np.float32)
    bf = lambda a: np.ascontiguousarray(a).astype(bfloat16)
    return bf(mA), bf(mB)


def build_in_maps(x, ln1_w, w_attn, w_proj, ln2_w, w_fc1, b_fc1, w_fc2, b_fc2):
    x = np.asarray(x, np.float32)
    B = x.shape[0]
    bf = lambda a: np.ascontiguousarray(np.asarray(a, np.float32)).astype(bfloat16)
    ln1 = np.asarray(ln1_w, np.float32)[:, None]
    ln2 = np.asarray(ln2_w, np.float32)[:, None]
    shared = dict(
        w_attn=bf(ln1 * np.asarray(w_attn, np.float32)),
        w_proj=bf(w_proj),
        w_fc1=bf(ln2 * np.asarray(w_fc1, np.float32)),
        w_fc2=bf(w_fc2),
        bfc1_col=np.ascontiguousarray(np.asarray(b_fc1, np.float32).reshape(F // P, P).T),
        bfc2_rep=np.tile(np.asarray(b_fc2, np.float32)[None, :], (P, 1)),
        ident=np.eye(P, dtype=np.float32).astype(bfloat16),
    )
    masks = [_masks(0), _masks(1)]
    in_maps = []
    for core in range(2 * B):
        s, par = core // 2, core % 2
        perm = _perm(par)
        xp = np.concatenate([x[s, c * CH:(c + 1) * CH] for c in perm], 0)
        mA, mB = masks[par]
        in_maps.append(dict(shared, x_seq=np.ascontiguousarray(xp),
                            maskA=mA, maskB=mB))
    return in_maps


def kernel(x, ln1_w, w_attn, w_proj, ln2_w, w_fc1, b_fc1, w_fc2, b_fc2):
    x = np.asarray(x, np.float32)
    B = x.shape[0]
    add_bfc1 = not np.allclose(b_fc1, 0.0)
    add_bfc2 = not np.allclose(b_fc2, 0.0)
    key = (add_bfc1, add_bfc2)
    if key not in _cache:
        _cache[key] = build_nc(*key)
    nc = _cache[key]

    in_maps = build_in_maps(x, ln1_w, w_attn, w_proj, ln2_w, w_fc1, b_fc1,
                            w_fc2, b_fc2)
    res = run_bass_kernel_spmd(nc, in_maps, list(range(2 * B)))
    out = np.empty_like(x)
    for core in range(2 * B):
        s, par = core // 2, core % 2
        perm = _perm(par)
        r = res.results[core]["out"]
        out[s, perm[0] * CH:(perm[0] + 1) * CH] = r[0:CH]
        out[s, perm[3] * CH:(perm[3] + 1) * CH] = r[CH:2 * CH]
    return out
